# revision 46
# baseline (speedup 1.0000x reference)
"""GCN (3-layer, PyG GCNConv-style) forward pass on 8 Trainium2 NeuronCores.

Strategy (dst-sharded graph parallelism):
  - Nodes are partitioned contiguously across the 8 cores (2560 per core,
    tiled into 20 tiles of 128 dst slots).
  - Per layer l:  Z = dis * (H @ Wl)  computed locally per core on its node
    shard (dis = deg^-1/2 absorbs the symmetric GCN normalization:
    out[d] = dis[d] * sum_e dis[src_e] * Z[src_e]).
  - AllGather replicates Z (bf16) to every core's HBM.  Each AllGather is
    split into three pieces by source-slot range so piece q's collective can
    fire as soon as its Z tiles are stored (mid-way through the previous
    layer's last phase), removing the layer-boundary bubble.  A tiny dummy
    AllGather at kernel start absorbs the CC cores' one-time mesh-init.
  - Each core gathers its incoming edges' source rows with the SWDGE
    dma_gather instruction (128 edges -> 128 SBUF partitions per block) and
    performs the segment-sum as a matmul with a 0/1 selector matrix
    (lhsT = selector [128 edges x 128 dst slots], rhs = messages
    [128 edges x feat]), accumulating blocks per dst tile in PSUM.
    Gathers are striped across the 4 SWDGE queues: queue q's descriptor
    generation runs on Q7 core pair (2q, 2q+1), so four gathers proceed
    concurrently (descgen is the dominant serial cost otherwise).
  - Aggregation runs in 3 phases (one per source piece); phases 0..P-2 fold
    dis into a running accumulator, the last phase applies bias and relu and
    immediately computes the next layer's Z per tile (PE transpose -> matmul
    with W), issuing the next layer's piece AllGathers as their tiles
    complete.

Edges (with self-loops appended) are grouped by (dst tile, src piece) on the
host and padded per group to a block multiple of 128; the block counts are
maxed across cores so all 8 cores execute an identical SPMD program.
Padding rows have a zero selector column and gather row 0 (finite * 0 = 0).
"""

import os
import sys

import numpy as np

sys.path.insert(0, "/opt/trn_rl_repo")

import ml_dtypes  # noqa: E402

import concourse.bass as bass  # noqa: E402
import concourse.bacc as bacc  # noqa: E402
import concourse.mybir as mybir  # noqa: E402
from concourse.bass_utils import run_bass_kernel_spmd  # noqa: E402
from concourse.library_config import mlp as _mlp_lib  # noqa: E402
from concourse.tile import TileContext  # noqa: E402
from concourse.tile_rust import add_dep_helper  # noqa: E402

BF16 = ml_dtypes.bfloat16
FP8 = ml_dtypes.float8_e4m3

# ----------------------------------------------------------------------------
# Problem configuration (hardcoded for nn_Encoder_17386027614431)
# ----------------------------------------------------------------------------
N_NODES = 20000
N_CORES = 8
T = 128          # dst slots per tile (= SBUF partitions)
NT = 20          # tiles per core
SHARD = NT * T   # 2560 node slots per core
D0 = 256                 # input feature dim
DL = [256, 128, 128]     # per-layer output dims (layer 3 padded 64 -> 128)
D3_REAL = 64
PIECE_TILES = [9, 6, 5]  # AllGather pieces (src-slot tile ranges)
NQ = 4                   # SWDGE queues
NCHUNK = 10              # gather chunks per phase


def _piece_bounds(nt=NT, pieces=None):
    if pieces is None:
        pieces = PIECE_TILES
    assert sum(pieces) == nt
    b = [0]
    for p in pieces:
        b.append(b[-1] + p)
    return b  # tile boundaries, len P+1


def _chunk_ranges(nt, nch):
    per = (nt + nch - 1) // nch
    return [(i, min(i + per, nt)) for i in range(0, nt, per)]


def _build_nc(BH, n_nodes=N_NODES, nt=NT, d0=D0, dl=None, d3_real=D3_REAL):
    """Build the SPMD Bass program.

    BH: [nt][P] list - number of 128-edge blocks per (dst tile, src piece),
    identical across cores."""
    if dl is None:
        dl = DL
    shard = nt * T
    pb = _piece_bounds(nt)
    P = len(pb) - 1
    psz = [(pb[q + 1] - pb[q]) * T for q in range(P)]  # slots per piece
    f32 = mybir.dt.float32
    bf16 = mybir.dt.bfloat16
    i16 = mybir.dt.int16
    mult = mybir.AluOpType.mult
    add = mybir.AluOpType.add
    relu = mybir.ActivationFunctionType.Relu

    # block offsets: piece-major, then tile
    boff = [[0] * P for _ in range(nt)]
    off = 0
    for q in range(P):
        for j in range(nt):
            boff[j][q] = off
            off += BH[j][q]
    totblk = off

    nc = bacc.Bacc("TRN2", num_devices=N_CORES, num_swdge_queues=NQ)

    # ---- kernel I/O ----
    xt = nc.dram_tensor("xt", [d0, shard], bf16, kind="ExternalInput")
    w1 = nc.dram_tensor("w1", [d0, dl[0]], bf16, kind="ExternalInput")
    w2 = nc.dram_tensor("w2", [dl[0], dl[1]], bf16, kind="ExternalInput")
    w3 = nc.dram_tensor("w3", [dl[1], dl[2]], bf16, kind="ExternalInput")
    brep1 = nc.dram_tensor("brep1", [T, dl[0]], f32, kind="ExternalInput")
    brep2 = nc.dram_tensor("brep2", [T, dl[1]], f32, kind="ExternalInput")
    brep3 = nc.dram_tensor("brep3", [T, dl[2]], f32, kind="ExternalInput")
    dis = nc.dram_tensor("dis", [T, nt], f32, kind="ExternalInput")
    idx = nc.dram_tensor("idx", [T, totblk * 8], i16, kind="ExternalInput")
    sel = nc.dram_tensor("sel", [T, totblk * T], bf16, kind="ExternalInput")
    ident = nc.dram_tensor("ident", [T, T], bf16, kind="ExternalInput")
    out = nc.dram_tensor("out", [shard, d3_real], f32, kind="ExternalOutput")

    # ---- internal DRAM bounce buffers for the collectives (per layer/piece)
    agin = [[nc.dram_tensor(f"agin{l}_{q}", [psz[q], dl[l]], bf16)
             for q in range(P)] for l in range(3)]
    agout = [[nc.dram_tensor(f"agout{l}_{q}", [N_CORES * psz[q], dl[l]], bf16,
                             addr_space="Shared")
              for q in range(P)] for l in range(3)]
    warm_in = nc.dram_tensor("warm_in", [16, 8], bf16)
    warm_out = nc.dram_tensor("warm_out", [N_CORES * 16, 8], bf16,
                              addr_space="Shared")
    rg = [list(range(N_CORES))]

    w_dram = [w1, w2, w3]
    w_chunks = [d0 // T, dl[0] // T, dl[1] // T]
    brep_dram = [brep1, brep2, brep3]

    with TileContext(nc) as tc:
        nc.gpsimd.load_library(_mlp_lib)

        # Warm the CC mesh (first collective pays ~50us one-time init).
        nc.gpsimd.collective_compute(
            "AllGather",
            mybir.AluOpType.bypass,
            replica_groups=rg,
            ins=[warm_in.ap().opt()],
            outs=[warm_out.ap().opt()],
        )

        with (
            tc.tile_pool(name="const", bufs=1) as cpool,
            tc.tile_pool(name="gath", bufs=8) as gpool,
            tc.tile_pool(name="selp", bufs=8) as spool,
            tc.tile_pool(name="accp", bufs=1) as apool,
            tc.tile_pool(name="hp", bufs=2) as hpool,
            tc.tile_pool(name="htp", bufs=3) as htpool,
            tc.tile_pool(name="tmp", bufs=3) as tpool,
            tc.tile_pool(name="zbp", bufs=3) as zbpool,
            tc.tile_pool(name="ps_agg", bufs=2, space="PSUM") as ps_agg,
            tc.tile_pool(name="ps_t", bufs=2, space="PSUM") as ps_t,
            tc.tile_pool(name="ps_z", bufs=2, space="PSUM") as ps_z,
        ):
            # ---- load constants ----
            def load_const(dram_h, shape, dtype, view=None):
                t = cpool.tile(shape, dtype, tag=f"c_{dram_h.name}")
                src = dram_h.ap() if view is None else view
                nc.sync.dma_start(out=t[:, :], in_=src)
                return t

            def load_const_chunked(dram_h, inner, dtype):
                cs = dram_h.shape[0] // T
                t = cpool.tile([T, cs * inner], dtype, tag=f"c_{dram_h.name}")
                nc.sync.dma_start(
                    out=t.rearrange("p (c n) -> p c n", c=cs),
                    in_=dram_h.ap().rearrange("(c p) n -> p c n", p=T),
                )
                return t

            idx_sb = load_const(idx, [T, totblk * 8], i16)
            xt_sb = load_const_chunked(xt, shard, bf16)
            w_sb = [load_const_chunked(w_dram[l], dl[l], bf16) for l in range(3)]
            brep_sb = [load_const(brep_dram[l], [T, dl[l]], f32) for l in range(3)]
            dis_sb = load_const(dis, [T, nt], f32)
            ident_sb = load_const(ident, [T, T], bf16)

            out_v = out.ap().rearrange("(n p) d -> p n d", p=T)
            agin_v = [[agin[l][q].ap().rearrange("(n p) d -> p n d", p=T)
                       for q in range(P)] for l in range(3)]

            ag_insts = [[None] * P for _ in range(3)]
            agin_dmas = [[[] for _ in range(P)] for _ in range(3)]

            def piece_of_tile(j):
                for q in range(P):
                    if j < pb[q + 1]:
                        return q
                raise AssertionError

            def z_prescale_store(l, j, zp):
                """dis * psum -> bf16 -> DRAM agin[l][piece-of-j]."""
                zb = zbpool.tile([T, dl[l]], bf16, tag="zb")
                nc.vector.tensor_scalar(zb[:, :], zp, dis_sb[:, j:j + 1], None, mult)
                q = piece_of_tile(j)
                d = nc.sync.dma_start(
                    out=agin_v[l][q][:, j - pb[q], :], in_=zb[:, :]
                )
                agin_dmas[l][q].append(d)

            def issue_ag(l, q):
                cc = nc.gpsimd.collective_compute(
                    "AllGather",
                    mybir.AluOpType.bypass,
                    replica_groups=rg,
                    ins=[agin[l][q].ap().opt()],
                    outs=[agout[l][q].ap().opt()],
                )
                for d in agin_dmas[l][q]:
                    add_dep_helper(cc.ins, d.ins, reason=f"ag{l}.{q} after dmas")
                ag_insts[l][q] = cc

            # ---- layer 1 local Z' = dis * (x @ W1) ----
            for j in range(nt):
                zp = ps_z.tile([T, dl[0]], f32, tag="zpsum")
                for c in range(w_chunks[0]):
                    nc.tensor.matmul(
                        zp[:, :],
                        xt_sb[:, c * shard + j * T: c * shard + (j + 1) * T],
                        w_sb[0][:, c * dl[0]:(c + 1) * dl[0]],
                        start=(c == 0),
                        stop=(c == w_chunks[0] - 1),
                    )
                z_prescale_store(0, j, zp[:, :])
                for q in range(P):
                    if j == pb[q + 1] - 1:
                        issue_ag(0, q)

            # ---- aggregation layers ----
            for l in range(3):
                d_el = dl[l]
                last = l == 2
                acc = apool.tile([T, nt * d_el], f32, tag="acc")

                def do_gather(j0, j1, q, qn):
                    b0, b1 = boff[j0][q], boff[j1 - 1][q] + BH[j1 - 1][q]
                    nb = b1 - b0
                    gt = gpool.tile([T, nb * d_el], bf16, tag="gath")
                    gt3 = gt.rearrange("p (n d) -> p n d", d=d_el)
                    g = nc.gpsimd.dma_gather(
                        gt3,
                        agout[l][q].ap(),
                        idx_sb[:, b0 * 8:b1 * 8],
                        nb * T,
                        nb * T,
                        d_el,
                        single_packet=False,
                        queue_num=qn,
                    )
                    add_dep_helper(g.ins, ag_insts[l][q].ins,
                                   reason=f"gather{l}.{q} after ag")
                    st = spool.tile([T, nb * T], bf16, tag="sel")
                    st3 = st.rearrange("p (n d) -> p n d", d=T)
                    nc.sync.dma_start(
                        out=st[:, :], in_=sel[:, b0 * T:b1 * T]
                    )
                    return gt3, st3, b0

                for p in range(P):
                    lastp = p == P - 1
                    for ci, (j0, j1) in enumerate(_chunk_ranges(nt, NCHUNK)):
                        gt3, st3, b0 = do_gather(j0, j1, p, ci % NQ)
                        for j in range(j0, j1):
                            ps = ps_agg.tile([T, d_el], f32, tag="aggpsum")
                            nb_j = BH[j][p]
                            jb = boff[j][p] - b0
                            for b in range(nb_j):
                                nc.tensor.matmul(
                                    ps[:, :],
                                    st3[:, jb + b, :],
                                    gt3[:, jb + b, :],
                                    start=(b == 0),
                                    stop=(b == nb_j - 1),
                                )
                            aj = acc[:, j * d_el:(j + 1) * d_el]
                            dj = dis_sb[:, j:j + 1]
                            if p == 0:
                                # acc = ps * dis
                                nc.vector.tensor_scalar(aj, ps[:, :], dj, None,
                                                        mult)
                                continue
                            if not lastp:
                                # acc += ps * dis
                                nc.vector.scalar_tensor_tensor(
                                    aj, ps[:, :], dj, aj, mult, add)
                                continue
                            # final piece: t2 = ps*dis + acc + bias
                            u = tpool.tile([T, d_el], f32, tag="post0")
                            nc.vector.scalar_tensor_tensor(
                                u[:, :], ps[:, :], dj, aj, mult, add)
                            t2 = tpool.tile([T, d_el], f32, tag="post1")
                            nc.vector.tensor_tensor(
                                t2[:, :], u[:, :], brep_sb[l][:, :], add)
                            if last:
                                nc.sync.dma_start(
                                    out=out_v[:, j, :], in_=t2[:, :d3_real])
                                continue
                            h = hpool.tile([T, d_el], bf16, tag="h")
                            nc.scalar.activation(h[:, :], t2[:, :], relu)
                            ln = l + 1
                            cs = w_chunks[ln]
                            zp = ps_z.tile([T, dl[ln]], f32, tag="zpsum")
                            for c in range(cs):
                                tp = ps_t.tile([T, T], bf16, tag="tpsum")
                                nc.tensor.matmul(
                                    tp[:, :],
                                    h[:, c * T:(c + 1) * T],
                                    ident_sb[:, :],
                                    is_transpose=True,
                                )
                                htc = htpool.tile([T, T], bf16, tag="ht")
                                nc.vector.tensor_copy(htc[:, :], tp[:, :])
                                nc.tensor.matmul(
                                    zp[:, :],
                                    htc[:, :],
                                    w_sb[ln][:, c * dl[ln]:(c + 1) * dl[ln]],
                                    start=(c == 0),
                                    stop=(c == cs - 1),
                                )
                            z_prescale_store(ln, j, zp[:, :])
                            for q in range(P):
                                if j == pb[q + 1] - 1:
                                    issue_ag(ln, q)

    nc.compile()
    return nc


# ----------------------------------------------------------------------------
# Host-side preprocessing (index work + sharding)
# ----------------------------------------------------------------------------
def _balanced_node_order(deg, n_nodes, nt):
    """Assign nodes to (core, tile) buckets so per-bucket in-edge counts are
    near-equal: sort by degree desc, deal round-robin (snake) over buckets.
    Returns node_order[n_slots] (original node id per slot, -1 for pad) and
    new_pos[n_nodes] (slot of each node)."""
    n_buckets = N_CORES * nt
    slots_total = n_buckets * T
    by_deg = np.argsort(-deg, kind="stable")
    node_order = -np.ones(slots_total, np.int64)
    new_pos = np.zeros(n_nodes, np.int64)
    fill = np.zeros(n_buckets, np.int64)
    b = 0
    direction = 1
    for node in by_deg:
        node_order[b * T + fill[b]] = node
        new_pos[node] = b * T + fill[b]
        fill[b] += 1
        b += direction
        if b == n_buckets:
            b = n_buckets - 1
            direction = -1
        elif b < 0:
            b = 0
            direction = 1
    return node_order, new_pos


def _preprocess(edge_index, n_nodes=N_NODES, nt=NT):
    """Group (self-loop-augmented) edges by (dst tile, src piece) per core;
    pad each group to a multiple of 128, block counts maxed across cores.
    Returns per-core gather indices, selectors, dis, BH[nt][P], node_order."""
    shard = nt * T
    pb = _piece_bounds(nt)
    P = len(pb) - 1
    pstart = np.array([pb[q] * T for q in range(P + 1)])  # slot bounds
    psz = np.diff(pstart)
    src = np.asarray(edge_index[0], dtype=np.int64)
    dst = np.asarray(edge_index[1], dtype=np.int64)
    loop = np.arange(n_nodes, dtype=np.int64)
    src = np.concatenate([src, loop])
    dst = np.concatenate([dst, loop])

    deg = np.bincount(dst, minlength=n_nodes).astype(np.float64)
    dis_full = np.where(deg > 0, 1.0 / np.sqrt(deg), 0.0)

    node_order, new_pos = _balanced_node_order(deg, n_nodes, nt)

    dpos = new_pos[dst]
    spos = new_pos[src]
    core_of = dpos // shard
    tile_of = (dpos % shard) // T
    slot_of = dpos % T
    sslot = spos % shard
    piece_of = np.searchsorted(pstart, sslot, side="right") - 1  # 0..P-1
    # row index within the piece's gathered buffer
    row_of = (spos // shard) * psz[piece_of] + sslot - pstart[piece_of]

    # Dedup: edges with the same src row targeting the same (core, tile,
    # piece) group share one gathered row; the selector column gets one
    # entry per edge (possibly several dst slots, or weight 2 for true
    # multi-edges).  ~5% fewer gather descriptors/bytes.
    order = np.lexsort((row_of, tile_of, piece_of, core_of))
    row_s = row_of[order]
    core_s = core_of[order]
    tile_s = tile_of[order]
    slot_s = slot_of[order]
    piece_s = piece_of[order]

    grp = (core_s * P + piece_s) * nt + tile_s
    first = np.ones(len(grp), bool)
    first[1:] = (grp[1:] != grp[:-1]) | (row_s[1:] != row_s[:-1])
    uid = np.cumsum(first) - 1  # unique (grp, row) id in sorted order

    counts = np.zeros((N_CORES, nt, P), np.int64)
    np.add.at(counts, (core_s[first], tile_s[first], piece_s[first]), 1)
    bh = np.maximum(
        1, np.ceil(counts.max(axis=0) / T).astype(np.int64))  # [nt, P]
    BH = bh.tolist()

    # block offsets (piece-major), same as the builder
    boff = np.zeros((nt, P), np.int64)
    off = 0
    for q in range(P):
        for j in range(nt):
            boff[j][q] = off
            off += bh[j][q]
    totblk = int(off)

    grp_start = np.zeros(N_CORES * P * nt + 1, np.int64)
    np.add.at(grp_start, grp + 1, 1)
    grp_start = np.cumsum(grp_start)
    rank = uid - uid[grp_start[grp]]  # unique-rank within group

    pos = boff[tile_s, piece_s] * T + rank  # padded position within the core
    blk = pos // T
    lane = pos % T

    idx_cores, sel_cores, dis_cores = [], [], []
    KC = totblk * T
    for c in range(N_CORES):
        m = core_s == c
        idx_pad = np.zeros(KC, np.int16)
        idx_pad[pos[m]] = row_s[m].astype(np.int16)
        idx_wrapped = np.tile(
            idx_pad.reshape(KC // 16, 16).T, (8, 1)).astype(np.int16)
        idx_cores.append(np.ascontiguousarray(idx_wrapped))

        selc = np.zeros((totblk, T, T), np.float32)
        np.add.at(selc, (blk[m], lane[m], slot_s[m]), 1.0)
        sel_cores.append(
            np.ascontiguousarray(
                selc.transpose(1, 0, 2).reshape(T, totblk * T)).astype(BF16))

        slots = node_order[c * shard:(c + 1) * shard]
        dis_c = np.where(slots >= 0, dis_full[np.maximum(slots, 0)], 0.0)
        dis_cores.append(
            np.ascontiguousarray(dis_c.reshape(nt, T).T).astype(np.float32))

    return idx_cores, sel_cores, dis_cores, BH, node_order


def _make_in_maps(x, W1, b1, W2, b2, W3, b3, edge_index,
                  n_nodes=N_NODES, nt=NT, d0=D0, dl=None, d3_real=D3_REAL):
    if dl is None:
        dl = DL
    shard = nt * T
    idx_cores, sel_cores, dis_cores, BH, node_order = _preprocess(
        edge_index, n_nodes, nt)

    x = np.asarray(x, np.float32)
    W3p = np.zeros((dl[1], dl[2]), np.float32)
    W3p[:, :d3_real] = np.asarray(W3, np.float32)
    b3p = np.zeros(dl[2], np.float32)
    b3p[:d3_real] = np.asarray(b3, np.float32)

    w1b = np.asarray(W1, np.float32).astype(BF16)
    w2b = np.asarray(W2, np.float32).astype(BF16)
    w3b = W3p.astype(BF16)
    brep1 = np.tile(np.asarray(b1, np.float32), (T, 1))
    brep2 = np.tile(np.asarray(b2, np.float32), (T, 1))
    brep3 = np.tile(b3p, (T, 1))
    identity = np.eye(T, dtype=BF16)

    in_maps = []
    for c in range(N_CORES):
        slots = node_order[c * shard:(c + 1) * shard]
        xs = np.where((slots >= 0)[:, None], x[np.maximum(slots, 0)], 0.0)
        xs = xs.astype(np.float32)
        in_maps.append({
            "xt": np.ascontiguousarray(xs.T).astype(BF16),
            "w1": w1b, "w2": w2b, "w3": w3b,
            "brep1": brep1, "brep2": brep2, "brep3": brep3,
            "dis": dis_cores[c],
            "idx": idx_cores[c],
            "sel": sel_cores[c],
            "ident": identity,
        })
    return in_maps, BH, node_order


_NC_CACHE = {}


def kernel_with_results(x, W1, b1, W2, b2, W3, b3, edge_index, trace=False):
    in_maps, BH, node_order = _make_in_maps(
        x, W1, b1, W2, b2, W3, b3, edge_index)
    key = tuple(tuple(r) for r in BH)
    if key not in _NC_CACHE:
        _NC_CACHE[key] = _build_nc(BH)
    nc = _NC_CACHE[key]
    res = run_bass_kernel_spmd(
        nc, in_maps, core_ids=list(range(N_CORES)), trace=trace
    )
    rows = np.concatenate(
        [np.asarray(res.results[c]["out"]) for c in range(N_CORES)], axis=0)
    full = np.zeros((N_NODES, rows.shape[1]), np.float32)
    real = node_order >= 0
    full[node_order[real]] = rows[real]
    return full, res


def kernel(x, W1, b1, W2, b2, W3, b3, edge_index):
    full, _ = kernel_with_results(x, W1, b1, W2, b2, W3, b3, edge_index)
    return full


# revision 47
# speedup vs baseline: 1.0378x; 1.0378x over previous
"""GCN (3-layer, PyG GCNConv-style) forward pass on 8 Trainium2 NeuronCores.

Strategy (dst-sharded graph parallelism):
  - Nodes are partitioned contiguously across the 8 cores (2560 per core,
    tiled into 20 tiles of 128 dst slots).
  - Per layer l:  Z = dis * (H @ Wl)  computed locally per core on its node
    shard (dis = deg^-1/2 absorbs the symmetric GCN normalization:
    out[d] = dis[d] * sum_e dis[src_e] * Z[src_e]).
  - AllGather replicates Z (bf16) to every core's HBM.  Each AllGather is
    split into three pieces by source-slot range so piece q's collective can
    fire as soon as its Z tiles are stored (mid-way through the previous
    layer's last phase), removing the layer-boundary bubble.  A tiny dummy
    AllGather at kernel start absorbs the CC cores' one-time mesh-init.
  - Each core gathers its incoming edges' source rows with the SWDGE
    dma_gather instruction (128 edges -> 128 SBUF partitions per block) and
    performs the segment-sum as a matmul with a 0/1 selector matrix
    (lhsT = selector [128 edges x 128 dst slots], rhs = messages
    [128 edges x feat]), accumulating blocks per dst tile in PSUM.
    Gathers are striped across the 4 SWDGE queues: queue q's descriptor
    generation runs on Q7 core pair (2q, 2q+1), so four gathers proceed
    concurrently (descgen is the dominant serial cost otherwise).
  - Aggregation runs in 3 phases (one per source piece); phases 0..P-2 fold
    dis into a running accumulator, the last phase applies bias and relu and
    immediately computes the next layer's Z per tile (PE transpose -> matmul
    with W), issuing the next layer's piece AllGathers as their tiles
    complete.

Edges (with self-loops appended) are grouped by (dst tile, src piece) on the
host and padded per group to a block multiple of 128; the block counts are
maxed across cores so all 8 cores execute an identical SPMD program.
Padding rows have a zero selector column and gather row 0 (finite * 0 = 0).
"""

import os
import sys

import numpy as np

sys.path.insert(0, "/opt/trn_rl_repo")

import ml_dtypes  # noqa: E402

import concourse.bass as bass  # noqa: E402
import concourse.bacc as bacc  # noqa: E402
import concourse.mybir as mybir  # noqa: E402
from concourse.bass_utils import run_bass_kernel_spmd  # noqa: E402
from concourse.library_config import mlp as _mlp_lib  # noqa: E402
from concourse.tile import TileContext  # noqa: E402
from concourse.tile_rust import add_dep_helper  # noqa: E402

BF16 = ml_dtypes.bfloat16
FP8 = ml_dtypes.float8_e4m3

# ----------------------------------------------------------------------------
# Problem configuration (hardcoded for nn_Encoder_17386027614431)
# ----------------------------------------------------------------------------
N_NODES = 20000
N_CORES = 8
T = 128          # dst slots per tile (= SBUF partitions)
NT = 20          # tiles per core
SHARD = NT * T   # 2560 node slots per core
D0 = 256                 # input feature dim
DL = [256, 128, 128]     # per-layer output dims (layer 3 padded 64 -> 128)
D3_REAL = 64
PIECE_TILES = [9, 6, 5]  # AllGather pieces (src-slot tile ranges)
NQ = 4                   # SWDGE queues
NCHUNK = 8               # gather chunks per phase


def _piece_bounds(nt=NT, pieces=None):
    if pieces is None:
        pieces = PIECE_TILES
    assert sum(pieces) == nt
    b = [0]
    for p in pieces:
        b.append(b[-1] + p)
    return b  # tile boundaries, len P+1


def _chunk_ranges(nt, nch):
    per = (nt + nch - 1) // nch
    return [(i, min(i + per, nt)) for i in range(0, nt, per)]


def _build_nc(BH, n_nodes=N_NODES, nt=NT, d0=D0, dl=None, d3_real=D3_REAL):
    """Build the SPMD Bass program.

    BH: [nt][P] list - number of 128-edge blocks per (dst tile, src piece),
    identical across cores."""
    if dl is None:
        dl = DL
    shard = nt * T
    pb = _piece_bounds(nt)
    P = len(pb) - 1
    psz = [(pb[q + 1] - pb[q]) * T for q in range(P)]  # slots per piece
    f32 = mybir.dt.float32
    bf16 = mybir.dt.bfloat16
    i16 = mybir.dt.int16
    mult = mybir.AluOpType.mult
    add = mybir.AluOpType.add
    relu = mybir.ActivationFunctionType.Relu

    # block offsets: piece-major, then tile
    boff = [[0] * P for _ in range(nt)]
    off = 0
    for q in range(P):
        for j in range(nt):
            boff[j][q] = off
            off += BH[j][q]
    totblk = off

    nc = bacc.Bacc("TRN2", num_devices=N_CORES, num_swdge_queues=NQ)

    # ---- kernel I/O ----
    xt = nc.dram_tensor("xt", [d0, shard], bf16, kind="ExternalInput")
    w1 = nc.dram_tensor("w1", [d0, dl[0]], bf16, kind="ExternalInput")
    w2 = nc.dram_tensor("w2", [dl[0], dl[1]], bf16, kind="ExternalInput")
    w3 = nc.dram_tensor("w3", [dl[1], dl[2]], bf16, kind="ExternalInput")
    brep1 = nc.dram_tensor("brep1", [T, dl[0]], f32, kind="ExternalInput")
    brep2 = nc.dram_tensor("brep2", [T, dl[1]], f32, kind="ExternalInput")
    brep3 = nc.dram_tensor("brep3", [T, dl[2]], f32, kind="ExternalInput")
    dis = nc.dram_tensor("dis", [T, nt], f32, kind="ExternalInput")
    idx = nc.dram_tensor("idx", [T, totblk * 8], i16, kind="ExternalInput")
    sel = nc.dram_tensor("sel", [T, totblk * T], bf16, kind="ExternalInput")
    ident = nc.dram_tensor("ident", [T, T], bf16, kind="ExternalInput")
    out = nc.dram_tensor("out", [shard, d3_real], f32, kind="ExternalOutput")

    # ---- internal DRAM bounce buffers for the collectives (per layer/piece)
    agin = [[nc.dram_tensor(f"agin{l}_{q}", [psz[q], dl[l]], bf16)
             for q in range(P)] for l in range(3)]
    agout = [[nc.dram_tensor(f"agout{l}_{q}", [N_CORES * psz[q], dl[l]], bf16,
                             addr_space="Shared")
              for q in range(P)] for l in range(3)]
    warm_in = nc.dram_tensor("warm_in", [16, 8], bf16)
    warm_out = nc.dram_tensor("warm_out", [N_CORES * 16, 8], bf16,
                              addr_space="Shared")
    rg = [list(range(N_CORES))]

    w_dram = [w1, w2, w3]
    w_chunks = [d0 // T, dl[0] // T, dl[1] // T]
    brep_dram = [brep1, brep2, brep3]

    with TileContext(nc) as tc:
        nc.gpsimd.load_library(_mlp_lib)

        # Warm the CC mesh (first collective pays ~50us one-time init).
        nc.gpsimd.collective_compute(
            "AllGather",
            mybir.AluOpType.bypass,
            replica_groups=rg,
            ins=[warm_in.ap().opt()],
            outs=[warm_out.ap().opt()],
        )

        with (
            tc.tile_pool(name="const", bufs=1) as cpool,
            tc.tile_pool(name="gath", bufs=8) as gpool,
            tc.tile_pool(name="selp", bufs=8) as spool,
            tc.tile_pool(name="accp", bufs=1) as apool,
            tc.tile_pool(name="hp", bufs=2) as hpool,
            tc.tile_pool(name="htp", bufs=3) as htpool,
            tc.tile_pool(name="tmp", bufs=3) as tpool,
            tc.tile_pool(name="zbp", bufs=3) as zbpool,
            tc.tile_pool(name="ps_agg", bufs=2, space="PSUM") as ps_agg,
            tc.tile_pool(name="ps_t", bufs=2, space="PSUM") as ps_t,
            tc.tile_pool(name="ps_z", bufs=2, space="PSUM") as ps_z,
        ):
            # ---- load constants ----
            def load_const(dram_h, shape, dtype, view=None):
                t = cpool.tile(shape, dtype, tag=f"c_{dram_h.name}")
                src = dram_h.ap() if view is None else view
                nc.sync.dma_start(out=t[:, :], in_=src)
                return t

            def load_const_chunked(dram_h, inner, dtype):
                cs = dram_h.shape[0] // T
                t = cpool.tile([T, cs * inner], dtype, tag=f"c_{dram_h.name}")
                nc.sync.dma_start(
                    out=t.rearrange("p (c n) -> p c n", c=cs),
                    in_=dram_h.ap().rearrange("(c p) n -> p c n", p=T),
                )
                return t

            idx_sb = load_const(idx, [T, totblk * 8], i16)
            xt_sb = load_const_chunked(xt, shard, bf16)
            w_sb = [load_const_chunked(w_dram[l], dl[l], bf16) for l in range(3)]
            brep_sb = [load_const(brep_dram[l], [T, dl[l]], f32) for l in range(3)]
            dis_sb = load_const(dis, [T, nt], f32)
            ident_sb = load_const(ident, [T, T], bf16)

            out_v = out.ap().rearrange("(n p) d -> p n d", p=T)
            agin_v = [[agin[l][q].ap().rearrange("(n p) d -> p n d", p=T)
                       for q in range(P)] for l in range(3)]

            ag_insts = [[None] * P for _ in range(3)]
            agin_dmas = [[[] for _ in range(P)] for _ in range(3)]

            def piece_of_tile(j):
                for q in range(P):
                    if j < pb[q + 1]:
                        return q
                raise AssertionError

            def z_prescale_store(l, j, zp):
                """dis * psum -> bf16 -> DRAM agin[l][piece-of-j]."""
                zb = zbpool.tile([T, dl[l]], bf16, tag="zb")
                nc.vector.tensor_scalar(zb[:, :], zp, dis_sb[:, j:j + 1], None, mult)
                q = piece_of_tile(j)
                d = nc.sync.dma_start(
                    out=agin_v[l][q][:, j - pb[q], :], in_=zb[:, :]
                )
                agin_dmas[l][q].append(d)

            def issue_ag(l, q):
                cc = nc.gpsimd.collective_compute(
                    "AllGather",
                    mybir.AluOpType.bypass,
                    replica_groups=rg,
                    ins=[agin[l][q].ap().opt()],
                    outs=[agout[l][q].ap().opt()],
                )
                for d in agin_dmas[l][q]:
                    add_dep_helper(cc.ins, d.ins, reason=f"ag{l}.{q} after dmas")
                ag_insts[l][q] = cc

            # ---- layer 1 local Z' = dis * (x @ W1) ----
            for j in range(nt):
                zp = ps_z.tile([T, dl[0]], f32, tag="zpsum")
                for c in range(w_chunks[0]):
                    nc.tensor.matmul(
                        zp[:, :],
                        xt_sb[:, c * shard + j * T: c * shard + (j + 1) * T],
                        w_sb[0][:, c * dl[0]:(c + 1) * dl[0]],
                        start=(c == 0),
                        stop=(c == w_chunks[0] - 1),
                    )
                z_prescale_store(0, j, zp[:, :])
                for q in range(P):
                    if j == pb[q + 1] - 1:
                        issue_ag(0, q)

            # ---- aggregation layers ----
            for l in range(3):
                d_el = dl[l]
                last = l == 2
                acc = apool.tile([T, nt * d_el], f32, tag="acc")

                def do_gather(j0, j1, q, qn):
                    b0, b1 = boff[j0][q], boff[j1 - 1][q] + BH[j1 - 1][q]
                    nb = b1 - b0
                    gt = gpool.tile([T, nb * d_el], bf16, tag="gath")
                    gt3 = gt.rearrange("p (n d) -> p n d", d=d_el)
                    g = nc.gpsimd.dma_gather(
                        gt3,
                        agout[l][q].ap(),
                        idx_sb[:, b0 * 8:b1 * 8],
                        nb * T,
                        nb * T,
                        d_el,
                        single_packet=False,
                        queue_num=qn,
                    )
                    add_dep_helper(g.ins, ag_insts[l][q].ins,
                                   reason=f"gather{l}.{q} after ag")
                    st = spool.tile([T, nb * T], bf16, tag="sel")
                    st3 = st.rearrange("p (n d) -> p n d", d=T)
                    nc.sync.dma_start(
                        out=st[:, :], in_=sel[:, b0 * T:b1 * T]
                    )
                    return gt3, st3, b0

                for p in range(P):
                    lastp = p == P - 1
                    for ci, (j0, j1) in enumerate(_chunk_ranges(nt, NCHUNK)):
                        gt3, st3, b0 = do_gather(j0, j1, p, ci % NQ)
                        for j in range(j0, j1):
                            ps = ps_agg.tile([T, d_el], f32, tag="aggpsum")
                            nb_j = BH[j][p]
                            jb = boff[j][p] - b0
                            for b in range(nb_j):
                                nc.tensor.matmul(
                                    ps[:, :],
                                    st3[:, jb + b, :],
                                    gt3[:, jb + b, :],
                                    start=(b == 0),
                                    stop=(b == nb_j - 1),
                                )
                            aj = acc[:, j * d_el:(j + 1) * d_el]
                            dj = dis_sb[:, j:j + 1]
                            if p == 0:
                                # acc = ps * dis
                                nc.vector.tensor_scalar(aj, ps[:, :], dj, None,
                                                        mult)
                                continue
                            if not lastp:
                                # acc += ps * dis
                                nc.vector.scalar_tensor_tensor(
                                    aj, ps[:, :], dj, aj, mult, add)
                                continue
                            # final piece: t2 = ps*dis + acc + bias
                            u = tpool.tile([T, d_el], f32, tag="post0")
                            nc.vector.scalar_tensor_tensor(
                                u[:, :], ps[:, :], dj, aj, mult, add)
                            t2 = tpool.tile([T, d_el], f32, tag="post1")
                            nc.vector.tensor_tensor(
                                t2[:, :], u[:, :], brep_sb[l][:, :], add)
                            if last:
                                nc.sync.dma_start(
                                    out=out_v[:, j, :], in_=t2[:, :d3_real])
                                continue
                            h = hpool.tile([T, d_el], bf16, tag="h")
                            nc.scalar.activation(h[:, :], t2[:, :], relu)
                            ln = l + 1
                            cs = w_chunks[ln]
                            zp = ps_z.tile([T, dl[ln]], f32, tag="zpsum")
                            for c in range(cs):
                                tp = ps_t.tile([T, T], bf16, tag="tpsum")
                                nc.tensor.matmul(
                                    tp[:, :],
                                    h[:, c * T:(c + 1) * T],
                                    ident_sb[:, :],
                                    is_transpose=True,
                                )
                                htc = htpool.tile([T, T], bf16, tag="ht")
                                nc.vector.tensor_copy(htc[:, :], tp[:, :])
                                nc.tensor.matmul(
                                    zp[:, :],
                                    htc[:, :],
                                    w_sb[ln][:, c * dl[ln]:(c + 1) * dl[ln]],
                                    start=(c == 0),
                                    stop=(c == cs - 1),
                                )
                            z_prescale_store(ln, j, zp[:, :])
                            for q in range(P):
                                if j == pb[q + 1] - 1:
                                    issue_ag(ln, q)

    nc.compile()
    return nc


# ----------------------------------------------------------------------------
# Host-side preprocessing (index work + sharding)
# ----------------------------------------------------------------------------
def _balanced_node_order(deg, n_nodes, nt):
    """Assign nodes to (core, tile) buckets so per-bucket in-edge counts are
    near-equal: sort by degree desc, deal round-robin (snake) over buckets.
    Returns node_order[n_slots] (original node id per slot, -1 for pad) and
    new_pos[n_nodes] (slot of each node)."""
    n_buckets = N_CORES * nt
    slots_total = n_buckets * T
    by_deg = np.argsort(-deg, kind="stable")
    node_order = -np.ones(slots_total, np.int64)
    new_pos = np.zeros(n_nodes, np.int64)
    fill = np.zeros(n_buckets, np.int64)
    b = 0
    direction = 1
    for node in by_deg:
        node_order[b * T + fill[b]] = node
        new_pos[node] = b * T + fill[b]
        fill[b] += 1
        b += direction
        if b == n_buckets:
            b = n_buckets - 1
            direction = -1
        elif b < 0:
            b = 0
            direction = 1
    return node_order, new_pos


def _preprocess(edge_index, n_nodes=N_NODES, nt=NT):
    """Group (self-loop-augmented) edges by (dst tile, src piece) per core;
    pad each group to a multiple of 128, block counts maxed across cores.
    Returns per-core gather indices, selectors, dis, BH[nt][P], node_order."""
    shard = nt * T
    pb = _piece_bounds(nt)
    P = len(pb) - 1
    pstart = np.array([pb[q] * T for q in range(P + 1)])  # slot bounds
    psz = np.diff(pstart)
    src = np.asarray(edge_index[0], dtype=np.int64)
    dst = np.asarray(edge_index[1], dtype=np.int64)
    loop = np.arange(n_nodes, dtype=np.int64)
    src = np.concatenate([src, loop])
    dst = np.concatenate([dst, loop])

    deg = np.bincount(dst, minlength=n_nodes).astype(np.float64)
    dis_full = np.where(deg > 0, 1.0 / np.sqrt(deg), 0.0)

    node_order, new_pos = _balanced_node_order(deg, n_nodes, nt)

    dpos = new_pos[dst]
    spos = new_pos[src]
    core_of = dpos // shard
    tile_of = (dpos % shard) // T
    slot_of = dpos % T
    sslot = spos % shard
    piece_of = np.searchsorted(pstart, sslot, side="right") - 1  # 0..P-1
    # row index within the piece's gathered buffer
    row_of = (spos // shard) * psz[piece_of] + sslot - pstart[piece_of]

    # Dedup: edges with the same src row targeting the same (core, tile,
    # piece) group share one gathered row; the selector column gets one
    # entry per edge (possibly several dst slots, or weight 2 for true
    # multi-edges).  ~5% fewer gather descriptors/bytes.
    order = np.lexsort((row_of, tile_of, piece_of, core_of))
    row_s = row_of[order]
    core_s = core_of[order]
    tile_s = tile_of[order]
    slot_s = slot_of[order]
    piece_s = piece_of[order]

    grp = (core_s * P + piece_s) * nt + tile_s
    first = np.ones(len(grp), bool)
    first[1:] = (grp[1:] != grp[:-1]) | (row_s[1:] != row_s[:-1])
    uid = np.cumsum(first) - 1  # unique (grp, row) id in sorted order

    counts = np.zeros((N_CORES, nt, P), np.int64)
    np.add.at(counts, (core_s[first], tile_s[first], piece_s[first]), 1)
    bh = np.maximum(
        1, np.ceil(counts.max(axis=0) / T).astype(np.int64))  # [nt, P]
    BH = bh.tolist()

    # block offsets (piece-major), same as the builder
    boff = np.zeros((nt, P), np.int64)
    off = 0
    for q in range(P):
        for j in range(nt):
            boff[j][q] = off
            off += bh[j][q]
    totblk = int(off)

    grp_start = np.zeros(N_CORES * P * nt + 1, np.int64)
    np.add.at(grp_start, grp + 1, 1)
    grp_start = np.cumsum(grp_start)
    rank = uid - uid[grp_start[grp]]  # unique-rank within group

    pos = boff[tile_s, piece_s] * T + rank  # padded position within the core
    blk = pos // T
    lane = pos % T

    idx_cores, sel_cores, dis_cores = [], [], []
    KC = totblk * T
    for c in range(N_CORES):
        m = core_s == c
        idx_pad = np.zeros(KC, np.int16)
        idx_pad[pos[m]] = row_s[m].astype(np.int16)
        idx_wrapped = np.tile(
            idx_pad.reshape(KC // 16, 16).T, (8, 1)).astype(np.int16)
        idx_cores.append(np.ascontiguousarray(idx_wrapped))

        selc = np.zeros((totblk, T, T), np.float32)
        np.add.at(selc, (blk[m], lane[m], slot_s[m]), 1.0)
        sel_cores.append(
            np.ascontiguousarray(
                selc.transpose(1, 0, 2).reshape(T, totblk * T)).astype(BF16))

        slots = node_order[c * shard:(c + 1) * shard]
        dis_c = np.where(slots >= 0, dis_full[np.maximum(slots, 0)], 0.0)
        dis_cores.append(
            np.ascontiguousarray(dis_c.reshape(nt, T).T).astype(np.float32))

    return idx_cores, sel_cores, dis_cores, BH, node_order


def _make_in_maps(x, W1, b1, W2, b2, W3, b3, edge_index,
                  n_nodes=N_NODES, nt=NT, d0=D0, dl=None, d3_real=D3_REAL):
    if dl is None:
        dl = DL
    shard = nt * T
    idx_cores, sel_cores, dis_cores, BH, node_order = _preprocess(
        edge_index, n_nodes, nt)

    x = np.asarray(x, np.float32)
    W3p = np.zeros((dl[1], dl[2]), np.float32)
    W3p[:, :d3_real] = np.asarray(W3, np.float32)
    b3p = np.zeros(dl[2], np.float32)
    b3p[:d3_real] = np.asarray(b3, np.float32)

    w1b = np.asarray(W1, np.float32).astype(BF16)
    w2b = np.asarray(W2, np.float32).astype(BF16)
    w3b = W3p.astype(BF16)
    brep1 = np.tile(np.asarray(b1, np.float32), (T, 1))
    brep2 = np.tile(np.asarray(b2, np.float32), (T, 1))
    brep3 = np.tile(b3p, (T, 1))
    identity = np.eye(T, dtype=BF16)

    in_maps = []
    for c in range(N_CORES):
        slots = node_order[c * shard:(c + 1) * shard]
        xs = np.where((slots >= 0)[:, None], x[np.maximum(slots, 0)], 0.0)
        xs = xs.astype(np.float32)
        in_maps.append({
            "xt": np.ascontiguousarray(xs.T).astype(BF16),
            "w1": w1b, "w2": w2b, "w3": w3b,
            "brep1": brep1, "brep2": brep2, "brep3": brep3,
            "dis": dis_cores[c],
            "idx": idx_cores[c],
            "sel": sel_cores[c],
            "ident": identity,
        })
    return in_maps, BH, node_order


_NC_CACHE = {}


def kernel_with_results(x, W1, b1, W2, b2, W3, b3, edge_index, trace=False):
    in_maps, BH, node_order = _make_in_maps(
        x, W1, b1, W2, b2, W3, b3, edge_index)
    key = tuple(tuple(r) for r in BH)
    if key not in _NC_CACHE:
        _NC_CACHE[key] = _build_nc(BH)
    nc = _NC_CACHE[key]
    res = run_bass_kernel_spmd(
        nc, in_maps, core_ids=list(range(N_CORES)), trace=trace
    )
    rows = np.concatenate(
        [np.asarray(res.results[c]["out"]) for c in range(N_CORES)], axis=0)
    full = np.zeros((N_NODES, rows.shape[1]), np.float32)
    real = node_order >= 0
    full[node_order[real]] = rows[real]
    return full, res


def kernel(x, W1, b1, W2, b2, W3, b3, edge_index):
    full, _ = kernel_with_results(x, W1, b1, W2, b2, W3, b3, edge_index)
    return full
